# revision 18
# baseline (speedup 1.0000x reference)
"""Trainium2 Bass kernel for nn_Dumplicate_Removal (duplicate-removal attention).

Strategy (8 cores, 2 SPMD launches, no collectives):
  NEFF-1 (column-sharded): core c computes xT_c = relu(emb[rank] + W_vis.T@featT)
    [128, 256] (bf16, SBUF-only), then k/q/v PARTIAL products
    xT_c^T @ [Wk|Wq|Wv] col-slices -> kqvc [256, 2048] fp16 DMA'd out.
    Geometry weights gw for the core's 32-row block computed concurrently:
      - pair log-distances L_cdim [32, 256] built in (i, j)-row layout
        (contiguous DRAM bounce, no scattered transpose),
      - amplitude-phase fold (A sin + B cos = C sin(.+phi)) halves the sine
        count: z = a x L outer product via K=14 bf16 matmuls (3-way hi/med/lo
        bf16 splits of both a and L keep phase err ~1e-4 rad),
      - range reduction round+sub on vector, Sin on scalar (fp16 out),
      - contraction over the 128 (freq x cdim) rows via 32 selector matmuls
        (C (x) e_i) accumulating straight into the gpre [32, 256] psum bank
        shared with the separable w/h-ratio feature matmuls (g23).
  host: sums the 8 kqv partials, adds biases, zeroes gw diagonals (staging).
  NEFF-2 (row-sharded): vw = kT.q / sqrt(dk), att = exp(vw)*gw, row-normalize,
    feat = att @ v, sigmoid(relu(feat)@Wr + br) for the core's 32 rows.
"""
import sys

for _p in ("/opt/trn_rl_repo", "/root/.axon_site/_ro/trn_rl_repo"):
    if _p not in sys.path:
        sys.path.append(_p)

import numpy as np
import concourse.bass as bass
import concourse.mybir as mybir
import concourse.tile as tile
from concourse import bacc
from concourse.bass_utils import run_bass_kernel_spmd
from concourse.masks import make_identity

F32 = mybir.dt.float32
BF16 = mybir.dt.bfloat16
FP16 = mybir.dt.float16
I32 = mybir.dt.int32
AT = mybir.ActivationFunctionType
OP = mybir.AluOpType

N = 256          # proposals
DHO = 4096       # feature dim
DMM = 1024       # model dim
DKEY = 512       # key dim
NCORES = 8
R = N // NCORES      # 32 rows per core (attention shard)
C = DMM // NCORES    # 128 mm-columns per core (fv shard)
M = 64               # frequencies
NKT = DHO // 128     # 32 contraction tiles for fv
PI = float(np.pi)
TWO_PI = float(2 * np.pi)
BIGF = 12582912.0    # 1.5 * 2**23: (y + BIGF) - BIGF == round-to-nearest(y)
KZ = 14              # z-matmul contraction rows
ZCH = 1024           # z psum chunk (2 matmuls of 512, 2 psum banks)
NZC = R * N // ZCH   # 8 chunks


def build_neff1(skip=()):
    nc = bacc.Bacc("TRN2", target_bir_lowering=False, debug=False, num_devices=NCORES)
    featP = nc.dram_tensor("featP", [128, NKT * N], BF16, kind="ExternalInput")
    wvisP = nc.dram_tensor("wvisP", [128, NKT * C], BF16, kind="ExternalInput")
    embP = nc.dram_tensor("embP", [128, 2 * C], BF16, kind="ExternalInput")
    wkqvP = nc.dram_tensor("wkqvP", [128, 2048], BF16, kind="ExternalInput")
    p_in = nc.dram_tensor("p", [N], F32, kind="ExternalInput")
    roisT = nc.dram_tensor("roisT", [4, N], F32, kind="ExternalInput")
    roisloc = nc.dram_tensor("roisloc", [R, 4], F32, kind="ExternalInput")
    bg = nc.dram_tensor("bg", [1], F32, kind="ExternalInput")
    abz = nc.dram_tensor("abz", [KZ, 128], BF16, kind="ExternalInput")
    cwall = nc.dram_tensor("cwall", [128, R * R], FP16, kind="ExternalInput")
    aoffp = nc.dram_tensor("aoffp", [2, 128], F32, kind="ExternalInput")
    aoffq = nc.dram_tensor("aoffq", [2, 128], F32, kind="ExternalInput")
    onesz = nc.dram_tensor("onesz", [R * N], BF16, kind="ExternalInput")
    onesf = nc.dram_tensor("onesf", [2 * N], F32, kind="ExternalInput")
    colpack = nc.dram_tensor("colpack", [128, 11], F32, kind="ExternalInput")
    kqv_out = nc.dram_tensor("kqvc", [256, 2048], F32, kind="ExternalOutput")
    gwc_out = nc.dram_tensor("gwc", [R, N], F32, kind="ExternalOutput")

    with tile.TileContext(nc) as tc:
        with (
            tc.tile_pool(name="const", bufs=1) as cpool,
            tc.tile_pool(name="stream", bufs=3) as spool,
            tc.tile_pool(name="work", bufs=2) as wpool,
            tc.tile_pool(name="big", bufs=1) as bpool,
            tc.tile_pool(name="dram", bufs=1, space="DRAM") as dpool,
            tc.tile_pool(name="psA", bufs=1, space="PSUM") as psA,   # fv: 1 bank
            tc.tile_pool(name="psB", bufs=2, space="PSUM") as psB,   # kqv: 2
            tc.tile_pool(name="psZ", bufs=2, space="PSUM") as psZ,   # z: 4
            tc.tile_pool(name="psC", bufs=1, space="PSUM") as psC,   # seq: 1
        ):
            # ---------- small loads ----------
            cpk = cpool.tile([128, 11], F32)
            nc.gpsimd.dma_start(cpk[:], colpack[:])
            abz_sb = cpool.tile([KZ, 128], BF16)
            nc.gpsimd.dma_start(abz_sb[:], abz[:])
            cw_sb = cpool.tile([128, R * R], FP16)
            nc.gpsimd.dma_start(cw_sb[:], cwall[:])
            aoffp_sb = cpool.tile([2, 128], F32)
            nc.scalar.dma_start(aoffp_sb[:], aoffp[:])
            aoffq_sb = cpool.tile([2, 128], F32)
            nc.scalar.dma_start(aoffq_sb[:], aoffq[:])
            x1y1 = cpool.tile([2, N], F32)
            nc.sync.dma_start(x1y1[:], roisT[0:2, :])
            x2y2 = cpool.tile([2, N], F32)
            nc.sync.dma_start(x2y2[:], roisT[2:4, :])
            rloc = cpool.tile([R, 4], F32)
            nc.sync.dma_start(rloc[:], roisloc[:])
            embt = cpool.tile([128, 2 * C], BF16)
            nc.scalar.dma_start(embt[:], embP[:])
            wkqv = cpool.tile([128, 2048], BF16)
            nc.scalar.dma_start(wkqv[:], wkqvP[:])
            bgcol = cpool.tile([R, 1], F32)
            nc.gpsimd.dma_start(bgcol[:], bass.AP(bg, 0, [[0, R], [1, 1]]))

            # ---------- geometry stats (vector/scalar, early) ----------
            wh = cpool.tile([2, N], F32)
            nc.vector.tensor_sub(wh[:], x2y2[:], x1y1[:])
            nc.vector.tensor_scalar(wh[:], wh[:], 1e-10, None, OP.add)
            cxy = cpool.tile([2, N], F32)
            nc.vector.tensor_add(cxy[:], x2y2[:], x1y1[:])
            nc.vector.tensor_scalar(cxy[:], cxy[:], 0.5, None, OP.mult)
            lwh = cpool.tile([2, N], F32)
            nc.scalar.activation(lwh[:], wh[:], AT.Ln)

            whl = cpool.tile([R, 2], F32)
            nc.vector.tensor_sub(whl[:], rloc[:, 2:4], rloc[:, 0:2])
            nc.vector.tensor_scalar(whl[:], whl[:], 1e-10, None, OP.add)
            cxyl = cpool.tile([R, 2], F32)
            nc.vector.tensor_add(cxyl[:], rloc[:, 2:4], rloc[:, 0:2])
            nc.vector.tensor_scalar(cxyl[:], cxyl[:], 0.5, None, OP.mult)
            lwhl = cpool.tile([R, 2], F32)
            nc.scalar.activation(lwhl[:], whl[:], AT.Ln)

            # cx/cy rows -> DRAM -> [R, N] partition-broadcast reads
            cxy_d = dpool.tile([2 * N], F32, name="cxy_d")
            nc.sync.dma_start(cxy_d[0:N], cxy[0:1, :])
            nc.sync.dma_start(cxy_d[N:2 * N], cxy[1:2, :])
            cxb = {}
            for cdim in range(2):
                t = cpool.tile([R, N], F32, name=f"cxb{cdim}")
                nc.sync.dma_start(
                    t[:], bass.AP(cxy_d.tensor, cxy_d.offset + cdim * N,
                                  [[0, R], [1, N]]))
                cxb[cdim] = t

            # glocflat / gflat for the separable c2/c3 features
            gloc_d = dpool.tile([2 * R], F32, name="gloc_d")
            nc.gpsimd.dma_start(gloc_d[0:R], lwhl[:, 0:1])
            nc.gpsimd.dma_start(gloc_d[R:2 * R], lwhl[:, 1:2])
            glocflat = cpool.tile([2, 2 * R], F32)
            nc.gpsimd.dma_start(glocflat[0:1, :], gloc_d[:])
            nc.gpsimd.dma_start(glocflat[1:2, :], onesf[0:2 * R])
            gflat = cpool.tile([2, 2 * N], F32)
            nc.gpsimd.dma_start(gflat[0:1, :], lwh[:])
            nc.gpsimd.dma_start(gflat[1:2, :], onesf[0:2 * N])

            # ---------- PE: c2/c3 phase matmuls first (small, unblocks scalar)
            zq = psC.tile([128, 2 * N], F32, tag="pc", name="zq")
            nc.tensor.matmul(zq[:], aoffq_sb[:], gflat[:], start=True, stop=True)
            zl = psC.tile([128, 2 * R], F32, tag="pc", name="zl")
            nc.tensor.matmul(zl[:], aoffp_sb[:], glocflat[:], start=True, stop=True)
            rq = cpool.tile([128, 2 * N], F32)
            nc.vector.tensor_scalar(rq[:], zq[:], BIGF, -BIGF, OP.add, OP.add)
            fq = cpool.tile([128, 2 * N], F32)
            nc.vector.tensor_sub(fq[:], zq[:], rq[:])
            scq = cpool.tile([128, 2 * N], FP16)
            if "c23" in skip:
                nc.vector.memset(scq[:], 0.0)
            else:
                nc.scalar.activation(scq[:], fq[:], AT.Sin, scale=TWO_PI)
            rl_ = cpool.tile([128, 2 * R], F32)
            nc.vector.tensor_scalar(rl_[:], zl[:], BIGF, -BIGF, OP.add, OP.add)
            fl_ = cpool.tile([128, 2 * R], F32)
            nc.vector.tensor_sub(fl_[:], zl[:], rl_[:])
            scl = cpool.tile([128, 2 * R], F32)
            nc.scalar.activation(scl[:], fl_[:], AT.Sin, scale=TWO_PI)

            ab = {cdim: (cpk[0:64, 3 + 2 * cdim:4 + 2 * cdim],
                         cpk[0:64, 4 + 2 * cdim:5 + 2 * cdim]) for cdim in range(4)}
            t1 = cpool.tile([64, R], F32, name="cmb1")
            t2 = cpool.tile([64, R], F32, name="cmb2")
            p23 = {}
            for cdim in (2, 3):
                wsel = cdim - 2
                sin64 = scl[0:64, wsel * R:(wsel + 1) * R]
                cos64 = scl[64:128, wsel * R:(wsel + 1) * R]
                A, B = ab[cdim]
                dst = cpool.tile([128, R], FP16, name=f"p23_{cdim}")
                nc.vector.tensor_scalar(t1[:], sin64, A, None, OP.mult)
                nc.vector.tensor_scalar(t2[:], cos64, B, None, OP.mult)
                nc.vector.tensor_add(dst[0:64, :], t1[:], t2[:])
                nc.vector.tensor_scalar(t1[:], sin64, B, None, OP.mult)
                nc.vector.tensor_scalar(t2[:], cos64, A, None, OP.mult)
                nc.vector.tensor_sub(dst[64:128, :], t1[:], t2[:])
                p23[cdim] = dst

            # ---------- pair log-distance L tiles (gpsimd), 3-way bf16 split
            zerR = cpool.tile([R, N], F32)
            nc.vector.memset(zerR[:], 0.0)
            ld = dpool.tile([6 * R * N], BF16, name="ld")
            for cdim in range(2):
                lwcol = lwhl[:, cdim:cdim + 1]
                ccol = cxyl[:, cdim:cdim + 1]
                d_t = wpool.tile([R, N], F32, tag="d_t")
                if "lprep" in skip:
                    nc.vector.memset(d_t[:], 1.0)
                else:
                    nc.vector.tensor_scalar(d_t[:], cxb[cdim][:], ccol, None,
                                            OP.subtract)
                    nc.scalar.activation(d_t[:], d_t[:], AT.Abs)
                mask = wpool.tile([R, N], I32, tag="mask")
                nc.vector.tensor_scalar(mask[:], d_t[:], 0.0, None, OP.is_equal)
                lt = wpool.tile([R, N], F32, tag="lt")
                nc.scalar.activation(lt[:], d_t[:], AT.Ln)
                nc.vector.tensor_scalar(lt[:], lt[:], lwcol, None, OP.subtract)
                nc.vector.copy_predicated(lt[:], mask[:], zerR[:])
                l1 = wpool.tile([R, N], BF16, tag="l1")
                nc.vector.tensor_copy(l1[:], lt[:])
                r1 = wpool.tile([R, N], F32, tag="r1")
                nc.vector.scalar_tensor_tensor(r1[:], l1[:], -1.0, lt[:],
                                               OP.mult, OP.add)
                l2 = wpool.tile([R, N], BF16, tag="l2")
                nc.vector.tensor_copy(l2[:], r1[:])
                r2 = wpool.tile([R, N], F32, tag="r2")
                nc.vector.scalar_tensor_tensor(r2[:], l2[:], -1.0, r1[:],
                                               OP.mult, OP.add)
                l3 = wpool.tile([R, N], BF16, tag="l3")
                nc.vector.tensor_copy(l3[:], r2[:])
                if "ldw" not in skip:
                    for si, lsp in enumerate((l1, l2, l3)):
                        row = cdim * 3 + si
                        dst = bass.AP(ld.tensor, ld.offset + row * R * N,
                                      [[N, R], [1, N]])
                        nc.sync.dma_start(dst, lsp[:])

            # zrhs [KZ, 8192]: (l1,l2,l3,l1,l2,l1) x 2 cdims + 2 phi-ones rows
            zrhs = bpool.tile([KZ, R * N], BF16, name="zrhs")
            if "zrhs" not in skip:
                rowmap = [0, 1, 2, 0, 1, 0, 3, 4, 5, 3, 4, 3]
                for k, src_row in enumerate(rowmap):
                    nc.sync.dma_start(zrhs[k:k + 1, :],
                                      ld[src_row * R * N:(src_row + 1) * R * N])
                nc.sync.dma_start(
                    zrhs[12:14, :], bass.AP(onesz, 0, [[0, 2], [1, R * N]]))
            else:
                nc.vector.memset(zrhs[:], 0.0)

            # ---------- permutation matrix (vector, early) ----------
            prow = cpool.tile([128, N], F32)
            nc.sync.dma_start(prow[:], bass.AP(p_in, 0, [[0, 128], [1, N]]))
            iot32 = cpool.tile([128, N], I32)
            nc.gpsimd.iota(iot32[:], pattern=[[1, N]], base=0, channel_multiplier=0)
            iof = cpool.tile([128, N], F32)
            nc.vector.tensor_copy(iof[:], iot32[:])
            riot32 = cpool.tile([128, 1], I32)
            nc.gpsimd.iota(riot32[:], pattern=[[1, 1]], base=0, channel_multiplier=1)
            riof = cpool.tile([128, 1], F32)
            nc.vector.tensor_copy(riof[:], riot32[:])
            mperm = cpool.tile([128, 2 * N], BF16)
            for rb in range(2):
                pcol = cpk[:, rb:rb + 1]
                g_gt = wpool.tile([128, N], F32, tag="g_gt")
                nc.vector.tensor_scalar(g_gt[:], prow[:], pcol, None, OP.is_gt)
                g_eq = wpool.tile([128, N], F32, tag="g_eq")
                nc.vector.tensor_scalar(g_eq[:], prow[:], pcol, None, OP.is_equal)
                rcol = wpool.tile([128, 1], F32, tag="rcol")
                nc.vector.tensor_scalar(rcol[:], riof[:], float(rb * 128), None, OP.add)
                g_lt = wpool.tile([128, N], F32, tag="g_lt")
                nc.vector.tensor_scalar(g_lt[:], iof[:], rcol[:], None, OP.is_lt)
                nc.vector.tensor_mul(g_eq[:], g_eq[:], g_lt[:])
                nc.vector.tensor_add(g_gt[:], g_gt[:], g_eq[:])
                srank = wpool.tile([128, 1], F32, tag="srank")
                nc.vector.reduce_sum(srank[:], g_gt[:], axis=mybir.AxisListType.X)
                nc.vector.tensor_scalar(
                    mperm[:, rb * N:(rb + 1) * N], iof[:], srank[:], None, OP.is_equal
                )

            # ---------- fv stream + z pipeline interleaved on PE ----------
            fvps = psA.tile([C, N], F32, name="fvps")
            QD = NKT // 4
            s2 = bpool.tile([128, R * N], FP16, name="s2")
            gpre = psC.tile([R, N], F32, tag="pc", name="gpre")

            def fv_quarter(qd):
                eng = (nc.gpsimd, nc.sync, nc.gpsimd, nc.sync)[qd]
                fq_t = spool.tile([128, QD * N], BF16, tag="featq", bufs=2)
                eng.dma_start(fq_t[:], featP[:, qd * QD * N:(qd + 1) * QD * N])
                wq_t = spool.tile([128, QD * C], BF16, tag="wvisq", bufs=2)
                nc.sync.dma_start(wq_t[:], wvisP[:, qd * QD * C:(qd + 1) * QD * C])
                for k2 in range(QD):
                    nc.tensor.matmul(fvps[:], wq_t[:, k2 * C:(k2 + 1) * C],
                                     fq_t[:, k2 * N:(k2 + 1) * N],
                                     start=(qd == 0 and k2 == 0), stop=False)

            def z_chunk(chz):
                zps = psZ.tile([128, ZCH], F32, tag="z", name=f"z{chz}")
                for h in range(ZCH // 512):
                    col0 = chz * ZCH + h * 512
                    nc.tensor.matmul(zps[:, h * 512:(h + 1) * 512], abz_sb[:],
                                     zrhs[:, col0:col0 + 512],
                                     start=True, stop=True)
                rnd = wpool.tile([128, ZCH], F32, tag="rnd")
                nc.vector.tensor_scalar(rnd[:], zps[:], BIGF, -BIGF, OP.add, OP.add)
                frac = wpool.tile([128, ZCH], F32, tag="frac")
                nc.vector.tensor_sub(frac[:], zps[:], rnd[:])
                nc.scalar.activation(s2[:, chz * ZCH:(chz + 1) * ZCH], frac[:],
                                     AT.Sin, scale=TWO_PI)

            def mv_chunk(chz):
                for ii in range(ZCH // N):
                    i = chz * (ZCH // N) + ii
                    nc.tensor.matmul(gpre[:], cw_sb[:, i * R:(i + 1) * R],
                                     s2[:, i * N:(i + 1) * N],
                                     start=False, stop=(i == R - 1))

            ZON = "z" not in skip
            MVON = ZON and ("mv" not in skip)
            fv_quarter(0)
            fv_quarter(1)
            if ZON: z_chunk(0)
            fv_quarter(2)
            if ZON: z_chunk(1)
            # gpre accumulation group opener (g23), then deferred mv chunks
            nc.tensor.matmul(gpre[:], p23[2][:], scq[:, 0:N], start=True, stop=False)
            nc.tensor.matmul(gpre[:], p23[3][:], scq[:, N:2 * N], start=False,
                             stop=(not MVON))
            if MVON: mv_chunk(0)
            fv_quarter(3)
            if ZON: z_chunk(2)
            if MVON: mv_chunk(1)
            for rb in range(2):
                nc.tensor.matmul(
                    fvps[:], embt[:, rb * C:(rb + 1) * C], mperm[:, rb * N:(rb + 1) * N],
                    start=False, stop=(rb == 1),
                )
            xt = cpool.tile([C, N], BF16)
            nc.scalar.activation(xt[:], fvps[:], AT.Relu, bias=cpk[:, 2:3])
            if ZON: z_chunk(3)
            if MVON: mv_chunk(2)

            # ---------- k/q/v partial products out ----------
            KQON = "kqv" not in skip
            for half in range(2):
                for ch in range(4):
                    idx = half * 4 + ch
                    st = wpool.tile([128, 512], F32, tag="kqvs", bufs=4)
                    if KQON:
                        pkv = psB.tile([128, 512], F32, tag="kqv", name=f"pkv{idx}")
                        nc.tensor.matmul(pkv[:], xt[:, half * 128:(half + 1) * 128],
                                         wkqv[:, ch * 512:(ch + 1) * 512],
                                         start=True, stop=True)
                        if idx % 2 == 0:
                            nc.scalar.activation(st[:], pkv[:], AT.Identity)
                        else:
                            nc.vector.tensor_copy(st[:], pkv[:])
                    else:
                        nc.vector.memset(st[:], 0.0)
                    nc.gpsimd.dma_start(
                        kqv_out[half * 128:(half + 1) * 128,
                                ch * 512:(ch + 1) * 512], st[:])
                    if idx == 1:
                        if ZON: z_chunk(4)
                        if MVON: mv_chunk(3)
                    elif idx == 3:
                        if ZON: z_chunk(5)
                        if MVON: mv_chunk(4)
                    elif idx == 5:
                        if ZON: z_chunk(6)
                        if MVON: mv_chunk(5)

            if ZON: z_chunk(7)
            if MVON:
                mv_chunk(6)
                mv_chunk(7)

            gwt = cpool.tile([R, N], F32)
            nc.scalar.activation(gwt[:], gpre[:], AT.Relu, bias=bgcol[:])
            nc.sync.dma_start(gwc_out[:], gwt[:])
    nc.compile()
    return nc


def build_neff2():
    nc = bacc.Bacc("TRN2", target_bir_lowering=False, debug=False, num_devices=NCORES)
    kTl = nc.dram_tensor("kTl", [128, 4 * R], BF16, kind="ExternalInput")
    qT = nc.dram_tensor("qT", [128, 4 * N], BF16, kind="ExternalInput")
    vW = nc.dram_tensor("vW", [128, 2 * DMM], BF16, kind="ExternalInput")
    gwz = nc.dram_tensor("gwz", [R, N], F32, kind="ExternalInput")
    wr = nc.dram_tensor("wr", [DMM], F32, kind="ExternalInput")
    br = nc.dram_tensor("br", [1], F32, kind="ExternalInput")
    outc = nc.dram_tensor("outc", [R, 1], F32, kind="ExternalOutput")

    with tile.TileContext(nc) as tc:
        with (
            tc.tile_pool(name="const", bufs=1) as cpool,
            tc.tile_pool(name="ps", bufs=1, space="PSUM") as psp,
            tc.tile_pool(name="pst", bufs=2, space="PSUM") as pst,
        ):
            kt = cpool.tile([128, 4 * R], BF16)
            nc.sync.dma_start(kt[:], kTl[:])
            gw_t = cpool.tile([R, N], F32)
            nc.sync.dma_start(gw_t[:], gwz[:])
            brc = cpool.tile([R, 1], F32)
            nc.sync.dma_start(brc[:], bass.AP(br, 0, [[0, R], [1, 1]]))
            qt = cpool.tile([128, 4 * N], BF16)
            nc.gpsimd.dma_start(qt[:], qT[:])
            vt = cpool.tile([128, 2 * DMM], BF16)
            nc.sync.dma_start(vt[:, 0:DMM], vW[:, 0:DMM])
            nc.scalar.dma_start(vt[:, DMM:2 * DMM], vW[:, DMM:2 * DMM])
            wrb = cpool.tile([R, DMM], F32)
            nc.gpsimd.dma_start(wrb[:], bass.AP(wr, 0, [[0, R], [1, DMM]]))

            pvw = psp.tile([R, N], F32, name="pvw")
            for ob in range(4):
                nc.tensor.matmul(pvw[:], kt[:, ob * R:(ob + 1) * R],
                                 qt[:, ob * N:(ob + 1) * N],
                                 start=(ob == 0), stop=(ob == 3))
            e_t = cpool.tile([R, N], F32)
            nc.scalar.activation(e_t[:], pvw[:], AT.Exp,
                                 scale=float(1.0 / np.sqrt(DKEY)))
            att = cpool.tile([R, N], F32)
            nc.vector.tensor_mul(att[:], e_t[:], gw_t[:])
            rowsum = cpool.tile([R, 1], F32)
            nc.vector.reduce_sum(rowsum[:], att[:], axis=mybir.AxisListType.X)
            nc.vector.tensor_scalar(rowsum[:], rowsum[:], 1e-10, None, OP.add)
            recip = cpool.tile([R, 1], F32)
            nc.vector.reciprocal(recip[:], rowsum[:])
            attn = cpool.tile([R, N], F32)
            nc.vector.tensor_scalar(attn[:], att[:], recip[:], None, OP.mult)

            ident = cpool.tile([128, 128], F32)
            make_identity(nc, ident[:])
            attT = cpool.tile([128, 2 * R], BF16)
            for jb in range(2):
                ptp = pst.tile([128, R], F32, tag="tp", name=f"ptp{jb}")
                nc.tensor.transpose(ptp[:], attn[:, jb * 128:(jb + 1) * 128],
                                    ident[0:R, 0:R])
                nc.vector.tensor_copy(attT[:, jb * R:(jb + 1) * R], ptp[:])
            pf = psp.tile([R, DMM], F32, name="pf")
            for ch in range(2):
                for jb in range(2):
                    nc.tensor.matmul(pf[:, ch * 512:(ch + 1) * 512],
                                     attT[:, jb * R:(jb + 1) * R],
                                     vt[:, jb * DMM + ch * 512:jb * DMM + (ch + 1) * 512],
                                     start=(jb == 0), stop=(jb == 1))
            rl = cpool.tile([R, DMM], F32)
            nc.scalar.activation(rl[:], pf[:], AT.Relu)
            scr = cpool.tile([R, DMM], F32)
            nc.vector.tensor_mul(scr[:], rl[:], wrb[:])
            zt = cpool.tile([R, 1], F32)
            nc.vector.reduce_sum(zt[:], scr[:], axis=mybir.AxisListType.X)
            ov = cpool.tile([R, 1], F32)
            nc.scalar.activation(ov[:], zt[:], AT.Sigmoid, bias=brc[:])
            nc.sync.dma_start(outc[:], ov[:])
    nc.compile()
    return nc


_NC1 = None
_NC2 = None
TRACE = False
LAST_TIMES = []
LAST_RES = []


def _split3(x, bf):
    x1 = x.astype(bf).astype(np.float64)
    x2 = (x - x1).astype(bf).astype(np.float64)
    x3 = (x - x1 - x2).astype(bf).astype(np.float64)
    return x1, x2, x3


def kernel(feature_obj, highest_prob, rois_obj, emb_table, W_vis, b_vis,
           Wk, bk, Wq, bq, Wv, bv, Wg, bg, Wr, br):
    global _NC1, _NC2
    import ml_dtypes
    f32 = np.float32
    bf = ml_dtypes.bfloat16
    f16 = np.float16
    ca = np.ascontiguousarray

    featT = np.asarray(feature_obj, f32).T
    WvisT = np.asarray(W_vis, f32).T
    roisT = ca(np.asarray(rois_obj, f32).T)
    featP = ca(featT.reshape(NKT, 128, N).transpose(1, 0, 2)
               .reshape(128, NKT * N).astype(bf))
    # angles in revolutions
    alpha = (100.0 / (1000.0 ** (np.arange(M, dtype=np.float64) / M)) / (2 * np.pi))
    wg0 = np.asarray(Wg, np.float64)[0]
    hp = np.asarray(highest_prob, f32)

    # amplitude-phase fold for c0/c1: A sin(aL) + B cos(aL) = C sin(aL + phi)
    A01 = np.stack([wg0[0:64], wg0[128:192]])          # [cdim, m] sin coefs
    B01 = np.stack([wg0[64:128], wg0[192:256]])        # cos coefs
    Cmag = np.hypot(A01, B01)
    phi = np.arctan2(B01, A01) / (2 * np.pi)           # revolutions

    a1, a2_, a3 = _split3(alpha, bf)
    abz_m = np.zeros((KZ, 128))
    # pairing with zrhs rows (l1,l2,l3,l1,l2,l1): a1,a1,a1,a2,a2,a3
    for cdim in range(2):
        sl = slice(cdim * 64, (cdim + 1) * 64)
        for r, av in enumerate([a1, a1, a1, a2_, a2_, a3]):
            abz_m[cdim * 6 + r, sl] = av
    phi2 = np.concatenate([phi[0], phi[1]])
    p_hi = phi2.astype(bf).astype(np.float64)
    p_lo = (phi2 - p_hi).astype(bf).astype(np.float64)
    abz_m[12, :] = p_hi
    abz_m[13, :] = p_lo
    abz_m = ca(abz_m.astype(bf))

    Cw = np.concatenate([Cmag[0], Cmag[1]])
    cwall = np.zeros((128, R * R))
    for i in range(R):
        cwall[:, i * R + i] = Cw
    cwall = ca(cwall.astype(f16))

    # c2/c3 separable path constants (as baseline)
    alpha_f = alpha.astype(f32)
    alpha2 = np.concatenate([alpha_f, alpha_f])
    offp = np.concatenate([np.zeros(M), np.full(M, 0.25)]).astype(f32)
    offq = np.concatenate([np.full(M, 0.25), np.zeros(M)]).astype(f32)
    aoffp = ca(np.stack([alpha2, offp]))
    aoffq = ca(np.stack([alpha2, offq]))
    ab_cols = []
    for cdim in range(4):
        for half in range(2):
            col = np.zeros(128, f32)
            col[0:64] = wg0[cdim * 128 + half * 64:cdim * 128 + (half + 1) * 64]
            ab_cols.append(col)

    wkT = np.asarray(Wk, f32).T     # [1024, 512]
    wqT = np.asarray(Wq, f32).T
    wvT = np.asarray(Wv, f32).T     # [1024, 1024]
    wkqv_all = np.concatenate([wkT, wqT, wvT], axis=1)  # [1024, 2048]

    if _NC1 is None:
        import os
        _NC1 = build_neff1(tuple(
            x for x in os.environ.get("K_SKIP", "").split(",") if x))
    in1 = []
    for c in range(NCORES):
        wvisPc = ca(WvisT[:, c * C:(c + 1) * C].reshape(NKT, 128, C)
                    .transpose(1, 0, 2).reshape(128, NKT * C).astype(bf))
        embPc = ca(np.asarray(emb_table, f32)[:, c * C:(c + 1) * C]
                   .reshape(2, 128, C).transpose(1, 0, 2)
                   .reshape(128, 2 * C).astype(bf))
        colpack = ca(np.stack(
            [hp[0:128], hp[128:256], np.asarray(b_vis, f32)[c * C:(c + 1) * C]]
            + ab_cols, axis=1))
        in1.append(dict(
            featP=featP,
            wvisP=wvisPc,
            embP=embPc,
            wkqvP=ca(wkqv_all[c * C:(c + 1) * C, :].astype(bf)),
            p=hp,
            roisT=roisT,
            roisloc=ca(np.asarray(rois_obj, f32)[c * R:(c + 1) * R]),
            bg=ca(np.asarray(bg, f32)),
            abz=abz_m,
            cwall=cwall,
            aoffp=aoffp,
            aoffq=aoffq,
            onesz=np.ones(R * N, bf),
            onesf=np.ones(2 * N, f32),
            colpack=colpack,
        ))
    res1 = run_bass_kernel_spmd(_NC1, in1, list(range(NCORES)), trace=TRACE)
    if TRACE:
        LAST_TIMES.append(res1.exec_time_ns)
        LAST_RES.append(res1)

    # host: sum kqv partials, add biases, split k/q/v
    acc = np.zeros((256, 2048), f32)
    for c in range(NCORES):
        acc += res1.results[c]["kqvc"].astype(f32)
    k_full = acc[:, 0:512] + np.asarray(bk, f32)
    q_full = acc[:, 512:1024] + np.asarray(bq, f32)
    v_full = acc[:, 1024:2048] + np.asarray(bv, f32)
    gws = [res1.results[c]["gwc"].copy() for c in range(NCORES)]
    for c in range(NCORES):
        for i in range(R):
            gws[c][i, c * R + i] = 0.0

    if _NC2 is None:
        _NC2 = build_neff2()

    qTp = ca(q_full.T.reshape(4, 128, N).transpose(1, 0, 2)
             .reshape(128, 4 * N).astype(bf))
    vWp = ca(v_full.reshape(2, 128, DMM).transpose(1, 0, 2)
             .reshape(128, 2 * DMM).astype(bf))
    in2 = []
    for c in range(NCORES):
        kl = k_full[c * R:(c + 1) * R, :]           # [32, 512]
        kTlp = ca(kl.T.reshape(4, 128, R).transpose(1, 0, 2)
                  .reshape(128, 4 * R).astype(bf))
        in2.append(dict(
            kTl=kTlp, qT=qTp, vW=vWp,
            gwz=gws[c],
            wr=ca(np.asarray(Wr, f32)[0]),
            br=ca(np.asarray(br, f32)),
        ))
    res2 = run_bass_kernel_spmd(_NC2, in2, list(range(NCORES)), trace=TRACE)
    if TRACE:
        LAST_TIMES.append(res2.exec_time_ns)
        LAST_RES.append(res2)
    out = np.concatenate([res2.results[c]["outc"] for c in range(NCORES)], axis=0)
    return out.astype(f32)
